# revision 42
# baseline (speedup 1.0000x reference)
"""Single-head causal attention on 8 TRN2 NeuronCores.

Problem shapes (hardcoded): B=8, T=2048, C=1024, H=64, fp32 I/O.
    q = x @ Wq; k = x @ Wk; v = x @ Wv          (per batch element)
    wei = softmax(causal_mask(q @ k.T * C**-0.5))
    out = wei @ v
Sharding: pure data parallel - one batch element per core, no collectives.

Per-core algorithm (bf16 matmuls, fp32 PSUM accumulation):
  - host pre-transposes x into a quarter-major layout [128, q, half, cb,
    512] so every DMA chunk is one contiguous 4KB run per partition; per
    512-wide T-slice: [q;k] = [Wq|Wk].T @ x-quarter on PE.
  - V projection is 2x col-group paired: even c-blocks accumulate into
    v_ps[0:64] (col grp h0), odd into v_ps[64:128] (h64); even/odd
    matmuls run CONCURRENTLY on disjoint array column halves and the
    halves are summed during the DVE drains.
  - S^T row-packed: kT2 holds Tk-block pairs in the partition halves,
    qT2hi duplicates q into the hi half; h0 reads q from qkT (q rows
    only).  The two halves of an S pair run CONCURRENTLY (row groups
    h0/h64).  ALL drains/shuffles run on DVE straight from PSUM as
    merged strided copies; Pool only does the 16 diagonal masks.
  - exp is one ACT per pair tile over a COMPACT range: the hi half is
    packed at its bank start so diagonal pairs activate 384-896 cols
    instead of 1024.  P = exp(S/32), no max-subtraction; diagonal
    blocks masked 0/1 on Pool as standalone schedule items.
  - v1 = [v | 1] -> [num|den] share one accumulator.  v natural is
    recovered by row-packed identity matmuls (VTR) whose four outputs
    land in ONE ps_big tile (lo row-group -> bank 0 cols 0:128, hi ->
    bank 1 cols 512:640) so concurrent drains never share a bank.
  - EPILOGUE IS HOST-SIDE: the [65,512] av accumulator is copied f32 ->
    SBUF and DMA'd per-slice to a [65,T] output (2KB descriptors); the
    num/den divide and [H,T]->[T,H] transpose happen in numpy.  This
    removes all epilogue matmuls/reciprocals from the device and keeps
    num/den in fp32 end to end.
  - THE SCHEDULE IS A FLAT GLOBAL INTERLEAVE tuned so ScalarE exp
    (20 ACTs, ~20us) starves as little as possible: projections run as
    early as the input DMA allows, S pair tiles are emitted densely
    (ps_big rotates 2 bufs so S(n+2) waits exp(n)), AV/V/VTR/EPn fill
    the PE between them; slice 3 runs its masked diagonal pair tiles
    first so the final exp->AV->store tail has no Pool-mask hop.
    Emission position IS PE execution position (static per-engine
    order): moving S tiles far from their flat slot was measured
    SLOWER twice (rotation coupling).
  - 14 dummy warmup matmuls release the HAM clock gate (PE starts at
    1.2 GHz, reaches 2.4 only after ~3.4us of sustained activity) while
    the input DMAs stream; v1's memset is split so the warmup operand
    (v1[:,0:4]) is ready right after the preamble barrier.
  - HW-DGE queues carry only inputs + stores (16 DMA instructions);
    the two halves of each x quarter stream on the two queues
    concurrently (per-queue ~130-160 GB/s, aggregate ~280-300).
  - NOTE run-to-run variance: the device alternates between full clock
    and a ~1.2x power-state downclock (ACT 1113ns vs ~1340ns per
    [128,1024] exp); compare configs only at equal clock.
"""

import numpy as np
import ml_dtypes

import concourse.bass as bass
import concourse.mybir as mybir
import concourse.tile as tile
from concourse import bacc
from concourse.bass_utils import run_bass_kernel_spmd

B, T, C, H = 8, 2048, 1024, 64
NCB = C // 128          # 8 C-blocks
NT = T // 128           # 16 Tk-blocks of 128
NJ = T // 512           # 4 Tq-slices of 512
SCALE = float(C) ** -0.5  # 1/32

BF16 = mybir.dt.bfloat16
F32 = mybir.dt.float32
npbf16 = ml_dtypes.bfloat16


class Ctx:
    pass


def build_attention(nc: bass.Bass, tc: tile.TileContext, ctx):
    g = Ctx()
    g.nc = nc
    # quarter-major x layout: [p, quarter, c-half, cb-in-half, 512] so each
    # DMA chunk is one contiguous 4KB run per partition (vs 4x 1KB).
    xT_d = nc.dram_tensor("xT", [128, NJ, 2, 4, 512], BF16,
                          kind="ExternalInput").ap()
    wqk_d = nc.dram_tensor("wqk", [128, NCB, 128], BF16,
                           kind="ExternalInput").ap()
    wv_d = nc.dram_tensor("wv", [128, NCB, H], BF16,
                          kind="ExternalInput").ap()
    ident_d = nc.dram_tensor("idents", [128, 192], BF16,
                             kind="ExternalInput").ap()
    g.o65_d = nc.dram_tensor("o65", [65, T], F32, kind="ExternalOutput").ap()

    consts = ctx.enter_context(tc.tile_pool(name="consts", bufs=1))
    persist = ctx.enter_context(tc.tile_pool(name="persist", bufs=1))
    g.pts = ctx.enter_context(tc.tile_pool(name="pts", bufs=9))
    g.outts = ctx.enter_context(tc.tile_pool(name="outts", bufs=2))
    g.vstage = ctx.enter_context(tc.tile_pool(name="vstage", bufs=2))
    g.ps_big = ctx.enter_context(tc.tile_pool(name="ps_big", bufs=2,
                                              space="PSUM"))
    g.ps_av = ctx.enter_context(tc.tile_pool(name="ps_av", bufs=2,
                                             space="PSUM"))
    g.ps_mix = ctx.enter_context(tc.tile_pool(name="ps_mix", bufs=2,
                                              space="PSUM"))

    g.v1 = persist.tile([128, NT, H + 1], BF16, tag="v1")  # [v | 1]
    # split memset: warmup's operand region first (first DVE op after the
    # preamble barrier) so dummy matmuls start ~7.5us, then the rest.
    nc.vector.memset(g.v1[:, 0:4, :], 1.0)
    nc.vector.memset(g.v1[:, 4:NT, :], 1.0)

    # ---- input DMAs: minimal count on the two HW DGE queues, in
    # consumption order; both halves of each T-quarter stream concurrently.
    g.wqk_sb = consts.tile([128, NCB, 128], BF16, tag="wqk")
    g.xT_sb = persist.tile([128, NJ, 2, 4, 512], BF16, tag="xT")
    g.wv_sb = consts.tile([128, NCB, H], BF16, tag="wv")
    ident_sb = consts.tile([128, 192], BF16, tag="idents")
    # scalar: wqk-lo (tiny) then all x-hi quarters back to back.
    # sync: x0lo first (QK0 c0-3 gate), wqk-hi + wv + idents, x-lo quarters.
    nc.scalar.dma_start(out=g.wqk_sb[:, 0:4, :], in_=wqk_d[:, 0:4, :])
    nc.sync.dma_start(out=g.xT_sb[:, 0, 0], in_=xT_d[:, 0, 0])
    nc.scalar.dma_start(out=g.xT_sb[:, 0, 1], in_=xT_d[:, 0, 1])
    nc.sync.dma_start(out=g.wqk_sb[:, 4:8, :], in_=wqk_d[:, 4:8, :])
    nc.sync.dma_start(out=g.wv_sb, in_=wv_d)
    nc.sync.dma_start(out=ident_sb, in_=ident_d)
    for qa in range(1, 4):
        nc.sync.dma_start(out=g.xT_sb[:, qa, 0], in_=xT_d[:, qa, 0])
        nc.scalar.dma_start(out=g.xT_sb[:, qa, 1], in_=xT_d[:, qa, 1])

    g.i64_sb = ident_sb[:, 0:64]
    g.causal_sb = ident_sb[:, 64:192]

    g.qkT = persist.tile([64, T], BF16, tag="qkT")       # q rows only
    g.qT2hi = persist.tile([128, T], BF16, tag="qT2hi")  # q in rows 64:128
    g.kT2 = persist.tile([128, T // 2], BF16, tag="kT2")
    g.vT = persist.tile([64, T], BF16, tag="vT")
    g.vT2 = persist.tile([128, T // 2], BF16, tag="vT2")

    g.s_pend = [[] for _ in range(NJ)]
    g.avs = [None] * NJ

    # ---- flat global schedule ------------------------------------------
    QK, V, S, M, VTR, AV, EP = (emit_qk, emit_v, emit_s, emit_mask,
                                emit_vtr, emit_avu, emit_epn)
    QK(g, 0, warmup=16)
    V(g, 0)
    S(g, 0, 0)
    S(g, 0, 1)
    QK(g, 1)
    M(g, 0, 0)
    M(g, 0, 1)
    VTR(g, 0)
    S(g, 1, 0)
    S(g, 1, 1)
    V(g, 1)
    AV(g, 0, 0)
    S(g, 1, 2)
    AV(g, 0, 1)
    QK(g, 2)
    VTR(g, 1)
    S(g, 1, 3)
    QK(g, 3)
    EP(g, 0, nc.sync)
    AV(g, 1, 0)
    AV(g, 1, 1)
    S(g, 2, 0)
    S(g, 2, 1)
    M(g, 1, 2)
    AV(g, 1, 2)
    M(g, 1, 3)
    AV(g, 1, 3)
    EP(g, 1, nc.scalar)
    V(g, 2)
    S(g, 2, 2)
    AV(g, 2, 0)
    S(g, 2, 3)
    AV(g, 2, 1)
    VTR(g, 2)
    S(g, 2, 4)
    AV(g, 2, 2)
    S(g, 3, 6)
    V(g, 3)
    M(g, 2, 4)
    AV(g, 2, 3)
    S(g, 2, 5)
    M(g, 3, 6)
    M(g, 2, 5)
    AV(g, 2, 4)
    AV(g, 2, 5)
    EP(g, 2, nc.sync)
    VTR(g, 3)
    S(g, 3, 7)
    AV(g, 3, 0)
    M(g, 3, 7)
    S(g, 3, 0)
    AV(g, 3, 1)
    S(g, 3, 1)
    AV(g, 3, 2)
    S(g, 3, 2)
    AV(g, 3, 3)
    S(g, 3, 3)
    AV(g, 3, 4)
    S(g, 3, 4)
    AV(g, 3, 5)
    S(g, 3, 5)
    AV(g, 3, 6)
    AV(g, 3, 7)
    EP(g, 3, nc.scalar)


def emit_qk(g, j, warmup=0):
    """[q;k] projection.  ALL drains run on DVE straight from PSUM:
    q -> qkT rows 0:64 and (shifted) qT2hi rows 64:128; k even blocks
    (shifted) -> kT2 lo, k odd blocks -> kT2 hi, merged as strided
    2-free-dim copies.  No Pool work at all."""
    nc = g.nc
    jsl = slice(j * 512, (j + 1) * 512)
    qk_ps = g.ps_mix.tile([128, 512], F32, tag="mix", name=f"qk_ps{j}")
    for w in range(warmup):  # HAM warmup; first real matmul resets PSUM
        nc.tensor.matmul(qk_ps[0:65, 0:260], lhsT=g.v1[:, 0, :],
                         rhs=g.v1[:, 0:4, :], start=True, stop=True,
                         skip_group_check=True)
    order = range(NCB) if j == 0 else list(range(4, 8)) + list(range(4))
    for ci, c in enumerate(order):
        nc.tensor.matmul(qk_ps, lhsT=g.wqk_sb[:, c, :],
                         rhs=g.xT_sb[:, j, c // 4, c % 4, :],
                         start=(ci == 0), stop=(ci == NCB - 1))
    c0 = j * 256
    nc.vector.tensor_copy(g.qkT[:, jsl], qk_ps[0:64, :])
    # even k-blocks (4j, 4j+2): PSUM hi -> kT2 lo half (partition shift)
    nc.vector.tensor_copy(
        g.kT2[0:64, c0:c0 + 256].rearrange("p (b n) -> p b n", b=2),
        qk_ps[64:128, :].rearrange("p (b h n) -> p b h n", b=2, h=2)[:, :, 0])
    nc.vector.tensor_copy(g.qT2hi[64:128, jsl], qk_ps[0:64, :])
    # odd k-blocks (4j+1, 4j+3): PSUM hi -> kT2 hi half (aligned)
    nc.vector.tensor_copy(
        g.kT2[64:128, c0:c0 + 256].rearrange("p (b n) -> p b n", b=2),
        qk_ps[64:128, :].rearrange("p (b h n) -> p b h n", b=2, h=2)[:, :, 1])


def emit_v(g, j):
    """v projection with 2x col-group pairing: even c-blocks accumulate
    into v_ps[0:64] (col grp h0), odd into v_ps[64:128] (h64); consecutive
    even/odd matmuls run CONCURRENTLY on disjoint array column halves.
    The halves are summed on DVE during the drains."""
    nc = g.nc
    jsl = slice(j * 512, (j + 1) * 512)
    v_ps = g.ps_av.tile([128, 512], F32, tag="av", name=f"v_ps{j}")
    for ci in range(4):
        for par in range(2):
            c = 2 * ci + par
            nc.tensor.matmul(v_ps[64 * par:64 * par + 64, :],
                             lhsT=g.wv_sb[:, c, :],
                             rhs=g.xT_sb[:, j, c // 4, c % 4, :],
                             start=(ci == 0), stop=(ci == 3))
    # drains on DVE.  TensorTensor may read only ONE input from PSUM, so
    # stage the odd-c partial in SBUF first, then sum halves during the
    # cast + odd-block partition shift.
    vhi = g.vstage.tile([64, 512], BF16, tag="vhi", name=f"vhi{j}")
    nc.vector.tensor_copy(vhi, v_ps[64:128, :])
    nc.vector.tensor_tensor(out=g.vT[:, jsl], in0=v_ps[0:64, :],
                            in1=vhi, op=mybir.AluOpType.add)
    nc.vector.tensor_tensor(
        out=g.vT2[64:128, j * 256:j * 256 + 256].rearrange(
            "p (b n) -> p b n", b=2),
        in0=v_ps[0:64, :].rearrange("p (b h n) -> p b h n", b=2, h=2)[:, :, 1],
        in1=vhi.rearrange("p (b h n) -> p b h n", b=2, h=2)[:, :, 1],
        op=mybir.AluOpType.add)


def emit_s(g, j, m, split=False):
    """Row-packed S^T pair tile (k-blocks 2m, 2m+1)."""
    nc = g.nc
    sp2 = g.ps_big.tile([128, 1024], F32, tag="big", name=f"sp{j}_{m}")
    pt2 = g.pts.tile([128, 1024], BF16, tag="pt", name=f"pt{j}_{m}")
    halves = []
    for half_idx, i in ((0, 2 * m), (1, 2 * m + 1)):
        n0 = max(0, i - 4 * j) * 128
        w = 512 - n0
        off = n0 if half_idx == 0 else 512  # hi half packed at bank start
        p0 = half_idx * 64
        rhs = (g.qkT if half_idx == 0 else g.qT2hi)
        nc.tensor.matmul(
            sp2[:, off:off + w],
            lhsT=g.kT2[p0:p0 + 64, m * 128:(m + 1) * 128],
            rhs=rhs[p0:p0 + 64, j * 512 + n0:(j + 1) * 512],
            start=True, stop=True)
        halves.append((off, n0))
    # exp over the compact range; cols in [512-n0_lo gap] are garbage.
    # split=True (first tile of a slice): separate lo/hi ACTs so the lo
    # half starts ~1.1us earlier than the full qT2hi/kT2o chain allows.
    lo0 = halves[0][0]
    hi_end = 512 + 512 - halves[1][1]
    if split:
        nc.scalar.activation(pt2[:, lo0:512], sp2[:, lo0:512],
                             mybir.ActivationFunctionType.Exp, scale=SCALE)
        nc.scalar.activation(pt2[:, 512:hi_end], sp2[:, 512:hi_end],
                             mybir.ActivationFunctionType.Exp, scale=SCALE)
    else:
        nc.scalar.activation(pt2[:, lo0:hi_end], sp2[:, lo0:hi_end],
                             mybir.ActivationFunctionType.Exp, scale=SCALE)
    g.s_pend[j].append((pt2, halves, 2 * m))


def emit_mask(g, j, m):
    """0/1 triangular mask on the diagonal blocks of pair tile (j, m).
    Standalone schedule item so Pool's head-of-line stays free."""
    nc = g.nc
    e = None
    for idx, (pt2, halves, i0) in enumerate(g.s_pend[j]):
        if i0 == 2 * m:
            e = idx
            break
    pt2, halves, i0 = g.s_pend[j][e]
    for half_idx, i in ((0, 2 * m), (1, 2 * m + 1)):
        if i - 4 * j >= 0:  # mask upper triangle of the diagonal block
            o = halves[half_idx][0]
            nc.gpsimd.tensor_mul(
                pt2[:, o:o + 128], pt2[:, o:o + 128], g.causal_sb)


def emit_vtr(g, j):
    """transpose v back into v1 = [v|1] via row-packed identity matmuls.
    All four outputs land in one ps_big tile: lo row-group -> bank 0
    (cols 0:128), hi row-group -> bank 1 (cols 512:640), so the two
    concurrent matmuls of a pair never drain into the same bank.
    (Same-bank concurrent lo/hi drains were tried and CRASH the device.)
    """
    nc = g.nc
    vp = g.ps_big.tile([128, 1024], F32, tag="big", name=f"vp{j}")
    for u, mt in enumerate((2 * j, 2 * j + 1)):
        tA, tB = 2 * mt, 2 * mt + 1
        nc.tensor.matmul(vp[:, u * 64:(u + 1) * 64],
                         lhsT=g.vT[:, tA * 128:(tA + 1) * 128],
                         rhs=g.i64_sb[0:64, :], start=True, stop=True)
        nc.tensor.matmul(vp[:, 512 + u * 64:512 + (u + 1) * 64],
                         lhsT=g.vT2[64:128, mt * 128:(mt + 1) * 128],
                         rhs=g.i64_sb[64:128, :], start=True, stop=True)
    dst = g.v1[:, 4 * j:4 * j + 4, 0:H].rearrange(
        "p (b o) h -> p b o h", b=2, o=2)
    nc.vector.tensor_copy(
        dst[:, :, 0], vp[:, 0:128].rearrange("p (b h) -> p b h", b=2))
    nc.vector.tensor_copy(
        dst[:, :, 1], vp[:, 512:640].rearrange("p (b h) -> p b h", b=2))


def emit_avu(g, j, e):
    """AV accumulation for the e-th EMITTED pair tile of slice j."""
    nc = g.nc
    if e == 0:
        g.avs[j] = g.ps_av.tile([65, 512], F32, tag="av", name=f"av{j}")
    av = g.avs[j]
    pt2, halves, i0 = g.s_pend[j][e]
    last = 2 * j + 1
    for d in range(2):
        off, n0 = halves[d]
        nc.tensor.matmul(av[:, n0:512], lhsT=g.v1[:, i0 + d, :],
                         rhs=pt2[:, off:off + 512 - n0],
                         start=(e == 0 and d == 0), stop=(e == last and d == 1))


def emit_epn(g, j, dq):
    """Per-slice epilogue: drain [num|den] f32 to SBUF and store.  The
    divide + transpose happen on the host."""
    nc = g.nc
    jsl = slice(j * 512, (j + 1) * 512)
    osb = g.outts.tile([65, 512], F32, tag="osb", name=f"osb{j}")
    nc.vector.tensor_copy(osb, g.avs[j])
    dq.dma_start(out=g.o65_d[:, jsl], in_=osb)


_CACHED = {}


def _get_nc(n=B):
    key = ("nc", n)
    if key not in _CACHED:
        from contextlib import ExitStack
        nc = bacc.Bacc("TRN2", target_bir_lowering=False, debug=False,
                       num_devices=n)
        with tile.TileContext(nc) as tc:
            with ExitStack() as ctx:
                build_attention(nc, tc, ctx)
        nc.compile()
        _CACHED[key] = nc
    return _CACHED[key]


def _quant_inputs(inputs, Wq, Wk, Wv):
    """Host-side prep: xT in [128, 8, T] bf16 layout, packed [Wq|Wk]."""
    inputs = np.asarray(inputs, dtype=np.float32)

    def wlayout(w, m):  # [C, m] -> [128, 8, m]
        return np.ascontiguousarray(
            np.asarray(w).astype(npbf16).reshape(8, 128, m).transpose(
                1, 0, 2))

    wqk = wlayout(np.concatenate([np.asarray(Wq), np.asarray(Wk)], axis=1),
                  128)
    wv = wlayout(Wv, H)

    idents = np.zeros((128, 192), dtype=npbf16)
    idents[0:64, 0:64] = np.eye(64, dtype=npbf16)
    idents[64:128, 0:64] = np.eye(64, dtype=npbf16)
    idents[:, 64:192] = np.triu(np.ones((128, 128), dtype=npbf16))

    in_maps = []
    for b in range(inputs.shape[0]):
        # [C,T] -> [p, quarter, c-half, cb-in-half, 512] (quarter-major so
        # each DMA chunk is one contiguous 4KB run per partition)
        xT = np.ascontiguousarray(
            inputs[b].T.astype(npbf16).reshape(2, 4, 128, 4, 512).transpose(
                2, 3, 0, 1, 4))
        in_maps.append({"xT": xT, "wqk": wqk, "wv": wv, "idents": idents})
    return in_maps


def _gather_out(res, n=B):
    """[65,T] per core -> [n,T,H]: host-side num/den divide + transpose."""
    outs = []
    for b in range(n):
        o65 = np.asarray(res.results[b]["o65"], dtype=np.float32)
        outs.append((o65[0:64] / o65[64:65]).T)
    return np.ascontiguousarray(np.stack(outs, axis=0).astype(np.float32))


def _spot_check(out, x, Wq, Wk, Wv):
    """Cheap host-side corruption detector: recompute one output row per
    128-row block per batch in fp32 numpy and compare.  The bf16 kernel
    sits at ~1e-2 per-row error; transient device corruption (observed
    ~2/50 executions after long run streaks: one all-NaN, one 2.5e-2
    global) blows individual rows far past 0.1."""
    wq = np.asarray(Wq, np.float32)
    wk = np.asarray(Wk, np.float32)
    wv = np.asarray(Wv, np.float32)
    scale = float(C) ** -0.5
    rows = np.arange(64, T, 128)
    for b in range(B):
        K = x[b] @ wk
        V = x[b] @ wv
        for t in rows:
            q = x[b, t] @ wq
            s = (K[: t + 1] @ q) * scale
            p = np.exp(s - s.max())
            p /= p.sum()
            ref = p @ V[: t + 1]
            err = np.linalg.norm(out[b, t] - ref) / np.linalg.norm(ref)
            if not np.isfinite(err) or err > 0.1:
                return False
    return True


def kernel(inputs, Wq, Wk, Wv):
    x = np.asarray(inputs, dtype=np.float32)
    in_maps = _quant_inputs(x, Wq, Wk, Wv)
    nc = _get_nc()
    for _attempt in range(3):
        res = run_bass_kernel_spmd(nc, in_maps, core_ids=list(range(B)))
        out = _gather_out(res)
        if _spot_check(out, x, Wq, Wk, Wv):
            break
    return out


# revision 43
# speedup vs baseline: 1.0004x; 1.0004x over previous
"""Single-head causal attention on 8 TRN2 NeuronCores.

Problem shapes (hardcoded): B=8, T=2048, C=1024, H=64, fp32 I/O.
    q = x @ Wq; k = x @ Wk; v = x @ Wv          (per batch element)
    wei = softmax(causal_mask(q @ k.T * C**-0.5))
    out = wei @ v
Sharding: pure data parallel - one batch element per core, no collectives.

Per-core algorithm (bf16 matmuls, fp32 PSUM accumulation):
  - host pre-transposes x into a quarter-major layout [128, q, half, cb,
    512] so every DMA chunk is one contiguous 4KB run per partition; per
    512-wide T-slice: [q;k] = [Wq|Wk].T @ x-quarter on PE.
  - V projection is 2x col-group paired: even c-blocks accumulate into
    v_ps[0:64] (col grp h0), odd into v_ps[64:128] (h64); even/odd
    matmuls run CONCURRENTLY on disjoint array column halves and the
    halves are summed during the DVE drains.
  - S^T row-packed: kT2 holds Tk-block pairs in the partition halves,
    qT2hi duplicates q into the hi half; h0 reads q from qkT (q rows
    only).  The two halves of an S pair run CONCURRENTLY (row groups
    h0/h64).  ALL drains/shuffles run on DVE straight from PSUM as
    merged strided copies; Pool only does the 16 diagonal masks.
  - exp is one ACT per pair tile over a COMPACT range: the hi half is
    packed at its bank start so diagonal pairs activate 384-896 cols
    instead of 1024.  P = exp(S/32), no max-subtraction; diagonal
    blocks masked 0/1 on Pool as standalone schedule items.
  - v1 = [v | 1] -> [num|den] share one accumulator.  v natural is
    recovered by row-packed identity matmuls (VTR) whose four outputs
    land in ONE ps_big tile (lo row-group -> bank 0 cols 0:128, hi ->
    bank 1 cols 512:640) so concurrent drains never share a bank.
  - EPILOGUE IS HOST-SIDE: the [65,512] av accumulator is copied f32 ->
    SBUF and DMA'd per-slice to a [65,T] output (2KB descriptors); the
    num/den divide and [H,T]->[T,H] transpose happen in numpy.  This
    removes all epilogue matmuls/reciprocals from the device and keeps
    num/den in fp32 end to end.
  - THE SCHEDULE IS A FLAT GLOBAL INTERLEAVE tuned so ScalarE exp
    (20 ACTs, ~20us) starves as little as possible: projections run as
    early as the input DMA allows, S pair tiles are emitted densely
    (ps_big rotates 2 bufs so S(n+2) waits exp(n)), AV/V/VTR/EPn fill
    the PE between them; slice 3 runs its masked diagonal pair tiles
    first so the final exp->AV->store tail has no Pool-mask hop.
    Emission position IS PE execution position (static per-engine
    order): moving S tiles far from their flat slot was measured
    SLOWER twice (rotation coupling).
  - 14 dummy warmup matmuls release the HAM clock gate (PE starts at
    1.2 GHz, reaches 2.4 only after ~3.4us of sustained activity) while
    the input DMAs stream; v1's memset is split so the warmup operand
    (v1[:,0:4]) is ready right after the preamble barrier.
  - HW-DGE queues carry only inputs + stores (16 DMA instructions);
    the two halves of each x quarter stream on the two queues
    concurrently (per-queue ~130-160 GB/s, aggregate ~280-300).
  - NOTE run-to-run variance: the device alternates between full clock
    and a ~1.2x power-state downclock (ACT 1113ns vs ~1340ns per
    [128,1024] exp); compare configs only at equal clock.
"""

import numpy as np
import ml_dtypes

import concourse.bass as bass
import concourse.mybir as mybir
import concourse.tile as tile
from concourse import bacc
from concourse.bass_utils import run_bass_kernel_spmd

B, T, C, H = 8, 2048, 1024, 64
NCB = C // 128          # 8 C-blocks
NT = T // 128           # 16 Tk-blocks of 128
NJ = T // 512           # 4 Tq-slices of 512
SCALE = float(C) ** -0.5  # 1/32

BF16 = mybir.dt.bfloat16
F32 = mybir.dt.float32
npbf16 = ml_dtypes.bfloat16


class Ctx:
    pass


def build_attention(nc: bass.Bass, tc: tile.TileContext, ctx):
    g = Ctx()
    g.nc = nc
    # quarter-major x layout: [p, quarter, c-half, cb-in-half, 512] so each
    # DMA chunk is one contiguous 4KB run per partition (vs 4x 1KB).
    xT_d = nc.dram_tensor("xT", [128, NJ, NCB, 512], BF16,
                          kind="ExternalInput").ap()
    wqk_d = nc.dram_tensor("wqk", [128, NCB, 128], BF16,
                           kind="ExternalInput").ap()
    wv_d = nc.dram_tensor("wv", [128, NCB, H], BF16,
                          kind="ExternalInput").ap()
    ident_d = nc.dram_tensor("idents", [128, 192], BF16,
                             kind="ExternalInput").ap()
    g.o65_d = nc.dram_tensor("o65", [65, T], F32, kind="ExternalOutput").ap()

    consts = ctx.enter_context(tc.tile_pool(name="consts", bufs=1))
    persist = ctx.enter_context(tc.tile_pool(name="persist", bufs=1))
    g.pts = ctx.enter_context(tc.tile_pool(name="pts", bufs=9))
    g.outts = ctx.enter_context(tc.tile_pool(name="outts", bufs=2))
    g.vstage = ctx.enter_context(tc.tile_pool(name="vstage", bufs=2))
    g.ps_big = ctx.enter_context(tc.tile_pool(name="ps_big", bufs=2,
                                              space="PSUM"))
    g.ps_av = ctx.enter_context(tc.tile_pool(name="ps_av", bufs=2,
                                             space="PSUM"))
    g.ps_mix = ctx.enter_context(tc.tile_pool(name="ps_mix", bufs=2,
                                              space="PSUM"))

    g.v1 = persist.tile([128, NT, H + 1], BF16, tag="v1")  # [v | 1]
    # split memset: warmup's operand region first (first DVE op after the
    # preamble barrier) so dummy matmuls start ~7.5us, then the rest.
    nc.vector.memset(g.v1[:, 0:4, :], 1.0)
    nc.vector.memset(g.v1[:, 4:NT, :], 1.0)

    # ---- input DMAs: minimal count on the two HW DGE queues, in
    # consumption order; both halves of each T-quarter stream concurrently.
    g.wqk_sb = consts.tile([128, NCB, 128], BF16, tag="wqk")
    g.xT_sb = persist.tile([128, NJ, NCB, 512], BF16, tag="xT")
    g.wv_sb = consts.tile([128, NCB, H], BF16, tag="wv")
    ident_sb = consts.tile([128, 192], BF16, tag="idents")
    # scalar: wqk-lo (tiny) then all x-hi quarters back to back.
    # sync: x0lo first (QK0 c0-3 gate), wqk-hi + wv + idents, x-lo quarters.
    nc.scalar.dma_start(out=g.wqk_sb, in_=wqk_d)
    nc.sync.dma_start(out=g.xT_sb[:, 0], in_=xT_d[:, 0])
    nc.scalar.dma_start(out=g.xT_sb[:, 1], in_=xT_d[:, 1])
    nc.sync.dma_start(out=g.wv_sb, in_=wv_d)
    nc.sync.dma_start(out=ident_sb, in_=ident_d)
    nc.sync.dma_start(out=g.xT_sb[:, 2], in_=xT_d[:, 2])
    nc.scalar.dma_start(out=g.xT_sb[:, 3], in_=xT_d[:, 3])

    g.i64_sb = ident_sb[:, 0:64]
    g.causal_sb = ident_sb[:, 64:192]

    g.qkT = persist.tile([64, T], BF16, tag="qkT")       # q rows only
    g.qT2hi = persist.tile([128, T], BF16, tag="qT2hi")  # q in rows 64:128
    g.kT2 = persist.tile([128, T // 2], BF16, tag="kT2")
    g.vT = persist.tile([64, T], BF16, tag="vT")
    g.vT2 = persist.tile([128, T // 2], BF16, tag="vT2")

    g.s_pend = [[] for _ in range(NJ)]
    g.avs = [None] * NJ

    # ---- flat global schedule ------------------------------------------
    QK, V, S, M, VTR, AV, EP = (emit_qk, emit_v, emit_s, emit_mask,
                                emit_vtr, emit_avu, emit_epn)
    QK(g, 0, warmup=16)
    V(g, 0)
    S(g, 0, 0)
    S(g, 0, 1)
    QK(g, 1)
    M(g, 0, 0)
    M(g, 0, 1)
    VTR(g, 0)
    S(g, 1, 0)
    S(g, 1, 1)
    V(g, 1)
    AV(g, 0, 0)
    S(g, 1, 2)
    AV(g, 0, 1)
    QK(g, 2)
    VTR(g, 1)
    S(g, 1, 3)
    QK(g, 3)
    EP(g, 0, nc.sync)
    AV(g, 1, 0)
    AV(g, 1, 1)
    S(g, 2, 0)
    S(g, 2, 1)
    M(g, 1, 2)
    AV(g, 1, 2)
    M(g, 1, 3)
    AV(g, 1, 3)
    EP(g, 1, nc.scalar)
    V(g, 2)
    S(g, 2, 2)
    AV(g, 2, 0)
    S(g, 2, 3)
    AV(g, 2, 1)
    VTR(g, 2)
    S(g, 2, 4)
    AV(g, 2, 2)
    S(g, 3, 6)
    V(g, 3)
    M(g, 2, 4)
    AV(g, 2, 3)
    S(g, 2, 5)
    M(g, 3, 6)
    M(g, 2, 5)
    AV(g, 2, 4)
    AV(g, 2, 5)
    EP(g, 2, nc.sync)
    VTR(g, 3)
    S(g, 3, 7)
    AV(g, 3, 0)
    M(g, 3, 7)
    S(g, 3, 0)
    AV(g, 3, 1)
    S(g, 3, 1)
    AV(g, 3, 2)
    S(g, 3, 2)
    AV(g, 3, 3)
    S(g, 3, 3)
    AV(g, 3, 4)
    S(g, 3, 4)
    AV(g, 3, 5)
    S(g, 3, 5)
    AV(g, 3, 6)
    AV(g, 3, 7)
    EP(g, 3, nc.scalar)


def emit_qk(g, j, warmup=0):
    """[q;k] projection.  ALL drains run on DVE straight from PSUM:
    q -> qkT rows 0:64 and (shifted) qT2hi rows 64:128; k even blocks
    (shifted) -> kT2 lo, k odd blocks -> kT2 hi, merged as strided
    2-free-dim copies.  No Pool work at all."""
    nc = g.nc
    jsl = slice(j * 512, (j + 1) * 512)
    qk_ps = g.ps_mix.tile([128, 512], F32, tag="mix", name=f"qk_ps{j}")
    for w in range(warmup):  # HAM warmup; first real matmul resets PSUM
        nc.tensor.matmul(qk_ps[0:65, 0:260], lhsT=g.v1[:, 0, :],
                         rhs=g.v1[:, 0:4, :], start=True, stop=True,
                         skip_group_check=True)
    order = range(NCB) if j == 0 else list(range(4, 8)) + list(range(4))
    for ci, c in enumerate(order):
        nc.tensor.matmul(qk_ps, lhsT=g.wqk_sb[:, c, :],
                         rhs=g.xT_sb[:, j, c, :],
                         start=(ci == 0), stop=(ci == NCB - 1))
    c0 = j * 256
    nc.vector.tensor_copy(g.qkT[:, jsl], qk_ps[0:64, :])
    # even k-blocks (4j, 4j+2): PSUM hi -> kT2 lo half (partition shift)
    nc.vector.tensor_copy(
        g.kT2[0:64, c0:c0 + 256].rearrange("p (b n) -> p b n", b=2),
        qk_ps[64:128, :].rearrange("p (b h n) -> p b h n", b=2, h=2)[:, :, 0])
    nc.vector.tensor_copy(g.qT2hi[64:128, jsl], qk_ps[0:64, :])
    # odd k-blocks (4j+1, 4j+3): PSUM hi -> kT2 hi half (aligned)
    nc.vector.tensor_copy(
        g.kT2[64:128, c0:c0 + 256].rearrange("p (b n) -> p b n", b=2),
        qk_ps[64:128, :].rearrange("p (b h n) -> p b h n", b=2, h=2)[:, :, 1])


def emit_v(g, j):
    """v projection with 2x col-group pairing: even c-blocks accumulate
    into v_ps[0:64] (col grp h0), odd into v_ps[64:128] (h64); consecutive
    even/odd matmuls run CONCURRENTLY on disjoint array column halves.
    The halves are summed on DVE during the drains."""
    nc = g.nc
    jsl = slice(j * 512, (j + 1) * 512)
    v_ps = g.ps_av.tile([128, 512], F32, tag="av", name=f"v_ps{j}")
    for ci in range(4):
        for par in range(2):
            c = 2 * ci + par
            nc.tensor.matmul(v_ps[64 * par:64 * par + 64, :],
                             lhsT=g.wv_sb[:, c, :],
                             rhs=g.xT_sb[:, j, c, :],
                             start=(ci == 0), stop=(ci == 3))
    # drains on DVE.  TensorTensor may read only ONE input from PSUM, so
    # stage the odd-c partial in SBUF first, then sum halves during the
    # cast + odd-block partition shift.
    vhi = g.vstage.tile([64, 512], BF16, tag="vhi", name=f"vhi{j}")
    nc.vector.tensor_copy(vhi, v_ps[64:128, :])
    nc.vector.tensor_tensor(out=g.vT[:, jsl], in0=v_ps[0:64, :],
                            in1=vhi, op=mybir.AluOpType.add)
    nc.vector.tensor_tensor(
        out=g.vT2[64:128, j * 256:j * 256 + 256].rearrange(
            "p (b n) -> p b n", b=2),
        in0=v_ps[0:64, :].rearrange("p (b h n) -> p b h n", b=2, h=2)[:, :, 1],
        in1=vhi.rearrange("p (b h n) -> p b h n", b=2, h=2)[:, :, 1],
        op=mybir.AluOpType.add)


def emit_s(g, j, m, split=False):
    """Row-packed S^T pair tile (k-blocks 2m, 2m+1)."""
    nc = g.nc
    sp2 = g.ps_big.tile([128, 1024], F32, tag="big", name=f"sp{j}_{m}")
    pt2 = g.pts.tile([128, 1024], BF16, tag="pt", name=f"pt{j}_{m}")
    halves = []
    for half_idx, i in ((0, 2 * m), (1, 2 * m + 1)):
        n0 = max(0, i - 4 * j) * 128
        w = 512 - n0
        off = n0 if half_idx == 0 else 512  # hi half packed at bank start
        p0 = half_idx * 64
        rhs = (g.qkT if half_idx == 0 else g.qT2hi)
        nc.tensor.matmul(
            sp2[:, off:off + w],
            lhsT=g.kT2[p0:p0 + 64, m * 128:(m + 1) * 128],
            rhs=rhs[p0:p0 + 64, j * 512 + n0:(j + 1) * 512],
            start=True, stop=True)
        halves.append((off, n0))
    # exp over the compact range; cols in [512-n0_lo gap] are garbage.
    # split=True (first tile of a slice): separate lo/hi ACTs so the lo
    # half starts ~1.1us earlier than the full qT2hi/kT2o chain allows.
    lo0 = halves[0][0]
    hi_end = 512 + 512 - halves[1][1]
    if split:
        nc.scalar.activation(pt2[:, lo0:512], sp2[:, lo0:512],
                             mybir.ActivationFunctionType.Exp, scale=SCALE)
        nc.scalar.activation(pt2[:, 512:hi_end], sp2[:, 512:hi_end],
                             mybir.ActivationFunctionType.Exp, scale=SCALE)
    else:
        nc.scalar.activation(pt2[:, lo0:hi_end], sp2[:, lo0:hi_end],
                             mybir.ActivationFunctionType.Exp, scale=SCALE)
    g.s_pend[j].append((pt2, halves, 2 * m))


def emit_mask(g, j, m):
    """0/1 triangular mask on the diagonal blocks of pair tile (j, m).
    Standalone schedule item so Pool's head-of-line stays free."""
    nc = g.nc
    e = None
    for idx, (pt2, halves, i0) in enumerate(g.s_pend[j]):
        if i0 == 2 * m:
            e = idx
            break
    pt2, halves, i0 = g.s_pend[j][e]
    for half_idx, i in ((0, 2 * m), (1, 2 * m + 1)):
        if i - 4 * j >= 0:  # mask upper triangle of the diagonal block
            o = halves[half_idx][0]
            nc.gpsimd.tensor_mul(
                pt2[:, o:o + 128], pt2[:, o:o + 128], g.causal_sb)


def emit_vtr(g, j):
    """transpose v back into v1 = [v|1] via row-packed identity matmuls.
    All four outputs land in one ps_big tile: lo row-group -> bank 0
    (cols 0:128), hi row-group -> bank 1 (cols 512:640), so the two
    concurrent matmuls of a pair never drain into the same bank.
    (Same-bank concurrent lo/hi drains were tried and CRASH the device.)
    """
    nc = g.nc
    vp = g.ps_big.tile([128, 1024], F32, tag="big", name=f"vp{j}")
    for u, mt in enumerate((2 * j, 2 * j + 1)):
        tA, tB = 2 * mt, 2 * mt + 1
        nc.tensor.matmul(vp[:, u * 64:(u + 1) * 64],
                         lhsT=g.vT[:, tA * 128:(tA + 1) * 128],
                         rhs=g.i64_sb[0:64, :], start=True, stop=True)
        nc.tensor.matmul(vp[:, 512 + u * 64:512 + (u + 1) * 64],
                         lhsT=g.vT2[64:128, mt * 128:(mt + 1) * 128],
                         rhs=g.i64_sb[64:128, :], start=True, stop=True)
    dst = g.v1[:, 4 * j:4 * j + 4, 0:H].rearrange(
        "p (b o) h -> p b o h", b=2, o=2)
    nc.vector.tensor_copy(
        dst[:, :, 0], vp[:, 0:128].rearrange("p (b h) -> p b h", b=2))
    nc.vector.tensor_copy(
        dst[:, :, 1], vp[:, 512:640].rearrange("p (b h) -> p b h", b=2))


def emit_avu(g, j, e):
    """AV accumulation for the e-th EMITTED pair tile of slice j."""
    nc = g.nc
    if e == 0:
        g.avs[j] = g.ps_av.tile([65, 512], F32, tag="av", name=f"av{j}")
    av = g.avs[j]
    pt2, halves, i0 = g.s_pend[j][e]
    last = 2 * j + 1
    for d in range(2):
        off, n0 = halves[d]
        nc.tensor.matmul(av[:, n0:512], lhsT=g.v1[:, i0 + d, :],
                         rhs=pt2[:, off:off + 512 - n0],
                         start=(e == 0 and d == 0), stop=(e == last and d == 1))


def emit_epn(g, j, dq):
    """Per-slice epilogue: drain [num|den] f32 to SBUF and store.  The
    divide + transpose happen on the host."""
    nc = g.nc
    jsl = slice(j * 512, (j + 1) * 512)
    osb = g.outts.tile([65, 512], F32, tag="osb", name=f"osb{j}")
    nc.vector.tensor_copy(osb, g.avs[j])
    dq.dma_start(out=g.o65_d[:, jsl], in_=osb)


_CACHED = {}


def _get_nc(n=B):
    key = ("nc", n)
    if key not in _CACHED:
        from contextlib import ExitStack
        nc = bacc.Bacc("TRN2", target_bir_lowering=False, debug=False,
                       num_devices=n)
        with tile.TileContext(nc) as tc:
            with ExitStack() as ctx:
                build_attention(nc, tc, ctx)
        nc.compile()
        _CACHED[key] = nc
    return _CACHED[key]


def _quant_inputs(inputs, Wq, Wk, Wv):
    """Host-side prep: xT in [128, 8, T] bf16 layout, packed [Wq|Wk]."""
    inputs = np.asarray(inputs, dtype=np.float32)

    def wlayout(w, m):  # [C, m] -> [128, 8, m]
        return np.ascontiguousarray(
            np.asarray(w).astype(npbf16).reshape(8, 128, m).transpose(
                1, 0, 2))

    wqk = wlayout(np.concatenate([np.asarray(Wq), np.asarray(Wk)], axis=1),
                  128)
    wv = wlayout(Wv, H)

    idents = np.zeros((128, 192), dtype=npbf16)
    idents[0:64, 0:64] = np.eye(64, dtype=npbf16)
    idents[64:128, 0:64] = np.eye(64, dtype=npbf16)
    idents[:, 64:192] = np.triu(np.ones((128, 128), dtype=npbf16))

    in_maps = []
    for b in range(inputs.shape[0]):
        # [C,T] -> [p, quarter, c-half, cb-in-half, 512] (quarter-major so
        # each DMA chunk is one contiguous 4KB run per partition)
        xT = np.ascontiguousarray(
            inputs[b].T.astype(npbf16).reshape(8, 128, 4, 512).transpose(
                1, 2, 0, 3))
        in_maps.append({"xT": xT, "wqk": wqk, "wv": wv, "idents": idents})
    return in_maps


def _gather_out(res, n=B):
    """[65,T] per core -> [n,T,H]: host-side num/den divide + transpose."""
    outs = []
    for b in range(n):
        o65 = np.asarray(res.results[b]["o65"], dtype=np.float32)
        outs.append((o65[0:64] / o65[64:65]).T)
    return np.ascontiguousarray(np.stack(outs, axis=0).astype(np.float32))


def _spot_check(out, x, Wq, Wk, Wv):
    """Cheap host-side corruption detector: recompute one output row per
    128-row block per batch in fp32 numpy and compare.  The bf16 kernel
    sits at ~1e-2 per-row error; transient device corruption (observed
    ~2/50 executions after long run streaks: one all-NaN, one 2.5e-2
    global) blows individual rows far past 0.1."""
    wq = np.asarray(Wq, np.float32)
    wk = np.asarray(Wk, np.float32)
    wv = np.asarray(Wv, np.float32)
    scale = float(C) ** -0.5
    rows = np.arange(64, T, 128)
    for b in range(B):
        K = x[b] @ wk
        V = x[b] @ wv
        for t in rows:
            q = x[b, t] @ wq
            s = (K[: t + 1] @ q) * scale
            p = np.exp(s - s.max())
            p /= p.sum()
            ref = p @ V[: t + 1]
            err = np.linalg.norm(out[b, t] - ref) / np.linalg.norm(ref)
            if not np.isfinite(err) or err > 0.1:
                return False
    return True


def kernel(inputs, Wq, Wk, Wv):
    x = np.asarray(inputs, dtype=np.float32)
    in_maps = _quant_inputs(x, Wq, Wk, Wv)
    nc = _get_nc()
    for _attempt in range(3):
        res = run_bass_kernel_spmd(nc, in_maps, core_ids=list(range(B)))
        out = _gather_out(res)
        if _spot_check(out, x, Wq, Wk, Wv):
            break
    return out


# revision 44
# speedup vs baseline: 1.0108x; 1.0104x over previous
"""Single-head causal attention on 8 TRN2 NeuronCores.

Problem shapes (hardcoded): B=8, T=2048, C=1024, H=64, fp32 I/O.
    q = x @ Wq; k = x @ Wk; v = x @ Wv          (per batch element)
    wei = softmax(causal_mask(q @ k.T * C**-0.5))
    out = wei @ v
Sharding: pure data parallel - one batch element per core, no collectives.

Per-core algorithm (bf16 matmuls, fp32 PSUM accumulation):
  - host pre-transposes x into a quarter-major layout [128, q, half, cb,
    512] so every DMA chunk is one contiguous 4KB run per partition; per
    512-wide T-slice: [q;k] = [Wq|Wk].T @ x-quarter on PE.
  - V projection is 2x col-group paired: even c-blocks accumulate into
    v_ps[0:64] (col grp h0), odd into v_ps[64:128] (h64); even/odd
    matmuls run CONCURRENTLY on disjoint array column halves and the
    halves are summed during the DVE drains.
  - S^T row-packed: kT2 holds Tk-block pairs in the partition halves,
    qT2hi duplicates q into the hi half; h0 reads q from qkT (q rows
    only).  The two halves of an S pair run CONCURRENTLY (row groups
    h0/h64).  ALL drains/shuffles run on DVE straight from PSUM as
    merged strided copies; Pool only does the 16 diagonal masks.
  - exp is one ACT per pair tile over a COMPACT range: the hi half is
    packed at its bank start so diagonal pairs activate 384-896 cols
    instead of 1024.  P = exp(S/32), no max-subtraction; diagonal
    blocks masked 0/1 on Pool as standalone schedule items.
  - v1 = [v | 1] -> [num|den] share one accumulator.  v natural is
    recovered by row-packed identity matmuls (VTR) whose four outputs
    land in ONE ps_big tile (lo row-group -> bank 0 cols 0:128, hi ->
    bank 1 cols 512:640) so concurrent drains never share a bank.
  - EPILOGUE IS HOST-SIDE: the [65,512] av accumulator is copied f32 ->
    SBUF and DMA'd per-slice to a [65,T] output (2KB descriptors); the
    num/den divide and [H,T]->[T,H] transpose happen in numpy.  This
    removes all epilogue matmuls/reciprocals from the device and keeps
    num/den in fp32 end to end.
  - THE SCHEDULE IS A FLAT GLOBAL INTERLEAVE tuned so ScalarE exp
    (20 ACTs, ~20us) starves as little as possible: projections run as
    early as the input DMA allows, S pair tiles are emitted densely
    (ps_big rotates 2 bufs so S(n+2) waits exp(n)), AV/V/VTR/EPn fill
    the PE between them; slice 3 runs its masked diagonal pair tiles
    first so the final exp->AV->store tail has no Pool-mask hop.
    Emission position IS PE execution position (static per-engine
    order): moving S tiles far from their flat slot was measured
    SLOWER twice (rotation coupling).
  - 14 dummy warmup matmuls release the HAM clock gate (PE starts at
    1.2 GHz, reaches 2.4 only after ~3.4us of sustained activity) while
    the input DMAs stream; v1's memset is split so the warmup operand
    (v1[:,0:4]) is ready right after the preamble barrier.
  - HW-DGE queues carry only inputs + stores (16 DMA instructions);
    the two halves of each x quarter stream on the two queues
    concurrently (per-queue ~130-160 GB/s, aggregate ~280-300).
  - NOTE run-to-run variance: the device alternates between full clock
    and a ~1.2x power-state downclock (ACT 1113ns vs ~1340ns per
    [128,1024] exp); compare configs only at equal clock.
"""

import numpy as np
import ml_dtypes

import concourse.bass as bass
import concourse.mybir as mybir
import concourse.tile as tile
from concourse import bacc
from concourse.bass_utils import run_bass_kernel_spmd

B, T, C, H = 8, 2048, 1024, 64
NCB = C // 128          # 8 C-blocks
NT = T // 128           # 16 Tk-blocks of 128
NJ = T // 512           # 4 Tq-slices of 512
SCALE = float(C) ** -0.5  # 1/32

BF16 = mybir.dt.bfloat16
F32 = mybir.dt.float32
npbf16 = ml_dtypes.bfloat16


class Ctx:
    pass


def build_attention(nc: bass.Bass, tc: tile.TileContext, ctx):
    g = Ctx()
    g.nc = nc
    # quarter-major x layout: [p, quarter, c-half, cb-in-half, 512] so each
    # DMA chunk is one contiguous 4KB run per partition (vs 4x 1KB).
    xT_d = nc.dram_tensor("xT", [128, NJ, 2, 4, 512], BF16,
                          kind="ExternalInput").ap()
    wqk_d = nc.dram_tensor("wqk", [128, NCB, 128], BF16,
                           kind="ExternalInput").ap()
    wv_d = nc.dram_tensor("wv", [128, NCB, H], BF16,
                          kind="ExternalInput").ap()
    ident_d = nc.dram_tensor("idents", [128, 192], BF16,
                             kind="ExternalInput").ap()
    g.o65_d = nc.dram_tensor("o65", [65, T], F32, kind="ExternalOutput").ap()

    consts = ctx.enter_context(tc.tile_pool(name="consts", bufs=1))
    persist = ctx.enter_context(tc.tile_pool(name="persist", bufs=1))
    g.pts = ctx.enter_context(tc.tile_pool(name="pts", bufs=9))
    g.outts = ctx.enter_context(tc.tile_pool(name="outts", bufs=2))
    g.vstage = ctx.enter_context(tc.tile_pool(name="vstage", bufs=2))
    g.ps_big = ctx.enter_context(tc.tile_pool(name="ps_big", bufs=2,
                                              space="PSUM"))
    g.ps_av = ctx.enter_context(tc.tile_pool(name="ps_av", bufs=2,
                                             space="PSUM"))
    g.ps_mix = ctx.enter_context(tc.tile_pool(name="ps_mix", bufs=2,
                                              space="PSUM"))

    g.v1 = persist.tile([128, NT, H + 1], BF16, tag="v1")  # [v | 1]
    # split memset: warmup's operand region first (first DVE op after the
    # preamble barrier) so dummy matmuls start ~7.5us, then the rest.
    nc.vector.memset(g.v1[:, 0:4, :], 1.0)
    nc.vector.memset(g.v1[:, 4:NT, :], 1.0)

    # ---- input DMAs: minimal count on the two HW DGE queues, in
    # consumption order; both halves of each T-quarter stream concurrently.
    g.wqk_sb = consts.tile([128, NCB, 128], BF16, tag="wqk")
    g.xT_sb = persist.tile([128, NJ, 2, 4, 512], BF16, tag="xT")
    g.wv_sb = consts.tile([128, NCB, H], BF16, tag="wv")
    ident_sb = consts.tile([128, 192], BF16, tag="idents")
    # scalar: wqk-lo (tiny) then all x-hi quarters back to back.
    # sync: x0lo first (QK0 c0-3 gate), wqk-hi + wv + idents, x-lo quarters.
    nc.scalar.dma_start(out=g.wqk_sb[:, 0:4, :], in_=wqk_d[:, 0:4, :])
    nc.sync.dma_start(out=g.xT_sb[:, 0, 0], in_=xT_d[:, 0, 0])
    nc.scalar.dma_start(out=g.xT_sb[:, 0, 1], in_=xT_d[:, 0, 1])
    nc.sync.dma_start(out=g.wqk_sb[:, 4:8, :], in_=wqk_d[:, 4:8, :])
    nc.sync.dma_start(out=g.wv_sb, in_=wv_d)
    nc.sync.dma_start(out=ident_sb, in_=ident_d)
    for qa in range(1, 4):
        nc.sync.dma_start(out=g.xT_sb[:, qa, 0], in_=xT_d[:, qa, 0])
        nc.scalar.dma_start(out=g.xT_sb[:, qa, 1], in_=xT_d[:, qa, 1])

    g.i64_sb = ident_sb[:, 0:64]
    g.causal_sb = ident_sb[:, 64:192]

    g.qkT = persist.tile([64, T], BF16, tag="qkT")       # q rows only
    g.qT2hi = persist.tile([128, T], BF16, tag="qT2hi")  # q in rows 64:128
    g.kT2 = persist.tile([128, T // 2], BF16, tag="kT2")
    g.vT = persist.tile([64, T], BF16, tag="vT")
    g.vT2 = persist.tile([128, T // 2], BF16, tag="vT2")

    g.s_pend = [[] for _ in range(NJ)]
    g.avs = [None] * NJ

    # ---- flat global schedule ------------------------------------------
    QK, V, S, M, VTR, AV, EP = (emit_qk, emit_v, emit_s, emit_mask,
                                emit_vtr, emit_avu, emit_epn)
    QK(g, 0, warmup=16)
    V(g, 0)
    S(g, 0, 0)
    S(g, 0, 1)
    QK(g, 1)
    M(g, 0, 0)
    M(g, 0, 1)
    VTR(g, 0)
    S(g, 1, 0)
    S(g, 1, 1)
    V(g, 1)
    AV(g, 0, 0)
    S(g, 1, 2)
    AV(g, 0, 1)
    QK(g, 2)
    VTR(g, 1)
    S(g, 1, 3)
    QK(g, 3)
    EP(g, 0, nc.sync)
    AV(g, 1, 0)
    AV(g, 1, 1)
    S(g, 2, 0)
    S(g, 2, 1)
    M(g, 1, 2)
    AV(g, 1, 2)
    M(g, 1, 3)
    AV(g, 1, 3)
    EP(g, 1, nc.scalar)
    V(g, 2)
    S(g, 2, 2)
    AV(g, 2, 0)
    S(g, 2, 3)
    AV(g, 2, 1)
    VTR(g, 2)
    S(g, 2, 4)
    AV(g, 2, 2)
    S(g, 3, 6)
    V(g, 3)
    M(g, 2, 4)
    AV(g, 2, 3)
    S(g, 2, 5)
    M(g, 3, 6)
    M(g, 2, 5)
    AV(g, 2, 4)
    AV(g, 2, 5)
    EP(g, 2, nc.sync)
    VTR(g, 3)
    S(g, 3, 7)
    AV(g, 3, 0)
    M(g, 3, 7)
    S(g, 3, 0)
    AV(g, 3, 1)
    S(g, 3, 1)
    AV(g, 3, 2)
    S(g, 3, 2)
    AV(g, 3, 3)
    S(g, 3, 3)
    AV(g, 3, 4)
    S(g, 3, 4)
    AV(g, 3, 5)
    S(g, 3, 5)
    AV(g, 3, 6)
    AV(g, 3, 7)
    EP(g, 3, nc.scalar)


def emit_qk(g, j, warmup=0):
    """[q;k] projection.  ALL drains run on DVE straight from PSUM:
    q -> qkT rows 0:64 and (shifted) qT2hi rows 64:128; k even blocks
    (shifted) -> kT2 lo, k odd blocks -> kT2 hi, merged as strided
    2-free-dim copies.  No Pool work at all."""
    nc = g.nc
    jsl = slice(j * 512, (j + 1) * 512)
    qk_ps = g.ps_mix.tile([128, 512], F32, tag="mix", name=f"qk_ps{j}")
    for w in range(warmup):  # HAM warmup; first real matmul resets PSUM
        nc.tensor.matmul(qk_ps[0:65, 0:260], lhsT=g.v1[:, 0, :],
                         rhs=g.v1[:, 0:4, :], start=True, stop=True,
                         skip_group_check=True)
    order = range(NCB) if j == 0 else list(range(4, 8)) + list(range(4))
    for ci, c in enumerate(order):
        nc.tensor.matmul(qk_ps, lhsT=g.wqk_sb[:, c, :],
                         rhs=g.xT_sb[:, j, c // 4, c % 4, :],
                         start=(ci == 0), stop=(ci == NCB - 1))
    c0 = j * 256
    nc.vector.tensor_copy(g.qkT[:, jsl], qk_ps[0:64, :])
    # even k-blocks (4j, 4j+2): PSUM hi -> kT2 lo half (partition shift)
    nc.vector.tensor_copy(
        g.kT2[0:64, c0:c0 + 256].rearrange("p (b n) -> p b n", b=2),
        qk_ps[64:128, :].rearrange("p (b h n) -> p b h n", b=2, h=2)[:, :, 0])
    nc.vector.tensor_copy(g.qT2hi[64:128, jsl], qk_ps[0:64, :])
    # odd k-blocks (4j+1, 4j+3): PSUM hi -> kT2 hi half (aligned)
    nc.vector.tensor_copy(
        g.kT2[64:128, c0:c0 + 256].rearrange("p (b n) -> p b n", b=2),
        qk_ps[64:128, :].rearrange("p (b h n) -> p b h n", b=2, h=2)[:, :, 1])


def emit_v(g, j):
    """v projection with 2x col-group pairing: even c-blocks accumulate
    into v_ps[0:64] (col grp h0), odd into v_ps[64:128] (h64); consecutive
    even/odd matmuls run CONCURRENTLY on disjoint array column halves.
    The halves are summed on DVE during the drains."""
    nc = g.nc
    jsl = slice(j * 512, (j + 1) * 512)
    v_ps = g.ps_av.tile([128, 512], F32, tag="av", name=f"v_ps{j}")
    for ci in range(4):
        for par in range(2):
            c = 2 * ci + par
            nc.tensor.matmul(v_ps[64 * par:64 * par + 64, :],
                             lhsT=g.wv_sb[:, c, :],
                             rhs=g.xT_sb[:, j, c // 4, c % 4, :],
                             start=(ci == 0), stop=(ci == 3))
    # drains on DVE.  TensorTensor may read only ONE input from PSUM, so
    # stage the odd-c partial in SBUF first, then sum halves during the
    # cast + odd-block partition shift.
    vhi = g.vstage.tile([64, 512], BF16, tag="vhi", name=f"vhi{j}")
    nc.vector.tensor_copy(vhi, v_ps[64:128, :])
    nc.vector.tensor_tensor(out=g.vT[:, jsl], in0=v_ps[0:64, :],
                            in1=vhi, op=mybir.AluOpType.add)
    nc.vector.tensor_tensor(
        out=g.vT2[64:128, j * 256:j * 256 + 256].rearrange(
            "p (b n) -> p b n", b=2),
        in0=v_ps[0:64, :].rearrange("p (b h n) -> p b h n", b=2, h=2)[:, :, 1],
        in1=vhi.rearrange("p (b h n) -> p b h n", b=2, h=2)[:, :, 1],
        op=mybir.AluOpType.add)


def emit_s(g, j, m, split=False):
    """Row-packed S^T pair tile (k-blocks 2m, 2m+1)."""
    nc = g.nc
    sp2 = g.ps_big.tile([128, 1024], F32, tag="big", name=f"sp{j}_{m}")
    pt2 = g.pts.tile([128, 1024], BF16, tag="pt", name=f"pt{j}_{m}")
    halves = []
    for half_idx, i in ((0, 2 * m), (1, 2 * m + 1)):
        n0 = max(0, i - 4 * j) * 128
        w = 512 - n0
        off = n0 if half_idx == 0 else 512  # hi half packed at bank start
        p0 = half_idx * 64
        rhs = (g.qkT if half_idx == 0 else g.qT2hi)
        nc.tensor.matmul(
            sp2[:, off:off + w],
            lhsT=g.kT2[p0:p0 + 64, m * 128:(m + 1) * 128],
            rhs=rhs[p0:p0 + 64, j * 512 + n0:(j + 1) * 512],
            start=True, stop=True)
        halves.append((off, n0))
    # exp over the compact range; cols in [512-n0_lo gap] are garbage.
    # split=True (first tile of a slice): separate lo/hi ACTs so the lo
    # half starts ~1.1us earlier than the full qT2hi/kT2o chain allows.
    lo0 = halves[0][0]
    hi_end = 512 + 512 - halves[1][1]
    if split:
        nc.scalar.activation(pt2[:, lo0:512], sp2[:, lo0:512],
                             mybir.ActivationFunctionType.Exp, scale=SCALE)
        nc.scalar.activation(pt2[:, 512:hi_end], sp2[:, 512:hi_end],
                             mybir.ActivationFunctionType.Exp, scale=SCALE)
    else:
        nc.scalar.activation(pt2[:, lo0:hi_end], sp2[:, lo0:hi_end],
                             mybir.ActivationFunctionType.Exp, scale=SCALE)
    g.s_pend[j].append((pt2, halves, 2 * m))


def emit_mask(g, j, m):
    """0/1 triangular mask on the diagonal blocks of pair tile (j, m).
    Standalone schedule item so Pool's head-of-line stays free."""
    nc = g.nc
    e = None
    for idx, (pt2, halves, i0) in enumerate(g.s_pend[j]):
        if i0 == 2 * m:
            e = idx
            break
    pt2, halves, i0 = g.s_pend[j][e]
    for half_idx, i in ((0, 2 * m), (1, 2 * m + 1)):
        if i - 4 * j >= 0:  # mask upper triangle of the diagonal block
            o = halves[half_idx][0]
            nc.gpsimd.tensor_mul(
                pt2[:, o:o + 128], pt2[:, o:o + 128], g.causal_sb)


def emit_vtr(g, j):
    """transpose v back into v1 = [v|1] via row-packed identity matmuls.
    All four outputs land in one ps_big tile: lo row-group -> bank 0
    (cols 0:128), hi row-group -> bank 1 (cols 512:640), so the two
    concurrent matmuls of a pair never drain into the same bank.
    (Same-bank concurrent lo/hi drains were tried and CRASH the device.)
    """
    nc = g.nc
    vp = g.ps_big.tile([128, 1024], F32, tag="big", name=f"vp{j}")
    for u, mt in enumerate((2 * j, 2 * j + 1)):
        tA, tB = 2 * mt, 2 * mt + 1
        nc.tensor.matmul(vp[:, u * 64:(u + 1) * 64],
                         lhsT=g.vT[:, tA * 128:(tA + 1) * 128],
                         rhs=g.i64_sb[0:64, :], start=True, stop=True)
        nc.tensor.matmul(vp[:, 512 + u * 64:512 + (u + 1) * 64],
                         lhsT=g.vT2[64:128, mt * 128:(mt + 1) * 128],
                         rhs=g.i64_sb[64:128, :], start=True, stop=True)
    dst = g.v1[:, 4 * j:4 * j + 4, 0:H].rearrange(
        "p (b o) h -> p b o h", b=2, o=2)
    nc.vector.tensor_copy(
        dst[:, :, 0], vp[:, 0:128].rearrange("p (b h) -> p b h", b=2))
    nc.vector.tensor_copy(
        dst[:, :, 1], vp[:, 512:640].rearrange("p (b h) -> p b h", b=2))


def emit_avu(g, j, e):
    """AV accumulation for the e-th EMITTED pair tile of slice j."""
    nc = g.nc
    if e == 0:
        g.avs[j] = g.ps_av.tile([65, 512], F32, tag="av", name=f"av{j}")
    av = g.avs[j]
    pt2, halves, i0 = g.s_pend[j][e]
    last = 2 * j + 1
    for d in range(2):
        off, n0 = halves[d]
        nc.tensor.matmul(av[:, n0:512], lhsT=g.v1[:, i0 + d, :],
                         rhs=pt2[:, off:off + 512 - n0],
                         start=(e == 0 and d == 0), stop=(e == last and d == 1))


def emit_epn(g, j, dq):
    """Per-slice epilogue: drain [num|den] f32 to SBUF and store.  The
    divide + transpose happen on the host."""
    nc = g.nc
    jsl = slice(j * 512, (j + 1) * 512)
    osb = g.outts.tile([65, 512], F32, tag="osb", name=f"osb{j}")
    nc.vector.tensor_copy(osb, g.avs[j])
    dq.dma_start(out=g.o65_d[:, jsl], in_=osb)


_CACHED = {}


def _get_nc(n=B):
    key = ("nc", n)
    if key not in _CACHED:
        from contextlib import ExitStack
        nc = bacc.Bacc("TRN2", target_bir_lowering=False, debug=False,
                       num_devices=n)
        with tile.TileContext(nc) as tc:
            with ExitStack() as ctx:
                build_attention(nc, tc, ctx)
        nc.compile()
        _CACHED[key] = nc
    return _CACHED[key]


def _quant_inputs(inputs, Wq, Wk, Wv):
    """Host-side prep: xT in [128, 8, T] bf16 layout, packed [Wq|Wk]."""
    inputs = np.asarray(inputs, dtype=np.float32)

    def wlayout(w, m):  # [C, m] -> [128, 8, m]
        return np.ascontiguousarray(
            np.asarray(w).astype(npbf16).reshape(8, 128, m).transpose(
                1, 0, 2))

    wqk = wlayout(np.concatenate([np.asarray(Wq), np.asarray(Wk)], axis=1),
                  128)
    wv = wlayout(Wv, H)

    idents = np.zeros((128, 192), dtype=npbf16)
    idents[0:64, 0:64] = np.eye(64, dtype=npbf16)
    idents[64:128, 0:64] = np.eye(64, dtype=npbf16)
    idents[:, 64:192] = np.triu(np.ones((128, 128), dtype=npbf16))

    in_maps = []
    for b in range(inputs.shape[0]):
        # [C,T] -> [p, quarter, c-half, cb-in-half, 512] (quarter-major so
        # each DMA chunk is one contiguous 4KB run per partition)
        xT = np.ascontiguousarray(
            inputs[b].T.astype(npbf16).reshape(2, 4, 128, 4, 512).transpose(
                2, 3, 0, 1, 4))
        in_maps.append({"xT": xT, "wqk": wqk, "wv": wv, "idents": idents})
    return in_maps


def _gather_out(res, n=B):
    """[65,T] per core -> [n,T,H]: host-side num/den divide + transpose."""
    outs = []
    for b in range(n):
        o65 = np.asarray(res.results[b]["o65"], dtype=np.float32)
        outs.append((o65[0:64] / o65[64:65]).T)
    return np.ascontiguousarray(np.stack(outs, axis=0).astype(np.float32))


def _spot_check(out, x, Wq, Wk, Wv):
    """Cheap host-side corruption detector: recompute one output row per
    128-row block per batch in fp32 numpy and compare.  The bf16 kernel
    sits at ~1e-2 per-row error; transient device corruption (observed
    ~2/50 executions after long run streaks: one all-NaN, one 2.5e-2
    global) blows individual rows far past 0.1."""
    wq = np.asarray(Wq, np.float32)
    wk = np.asarray(Wk, np.float32)
    wv = np.asarray(Wv, np.float32)
    scale = float(C) ** -0.5
    rows = np.arange(64, T, 128)
    for b in range(B):
        K = x[b] @ wk
        V = x[b] @ wv
        for t in rows:
            q = x[b, t] @ wq
            s = (K[: t + 1] @ q) * scale
            p = np.exp(s - s.max())
            p /= p.sum()
            ref = p @ V[: t + 1]
            err = np.linalg.norm(out[b, t] - ref) / np.linalg.norm(ref)
            if not np.isfinite(err) or err > 0.1:
                return False
    return True


def kernel(inputs, Wq, Wk, Wv):
    x = np.asarray(inputs, dtype=np.float32)
    in_maps = _quant_inputs(x, Wq, Wk, Wv)
    nc = _get_nc()
    for _attempt in range(3):
        res = run_bass_kernel_spmd(nc, in_maps, core_ids=list(range(B)))
        out = _gather_out(res)
        if _spot_check(out, x, Wq, Wk, Wv):
            break
    return out
